# revision 15
# baseline (speedup 1.0000x reference)
"""Trainium2 Bass kernel for nn_DLI_loss_2 (ragged_sequence).

Reference computation (B=16, S=4096, E=1024, T=32, H=512):
    states[b,k,:] = encoder_output[b, ids[b,k], :]          (ragged gather)
    ... 2-step LSTM -> h2 -> a = h2 @ w_h + fc_b            (per (b,j) scalar)
    t = states @ w_t                                        (w_t = fc_w[0, H:])
    logits[b,j,k] = a[b,j] + t[b,k]  masked to k >= j+2
    loss = mean_j( logsumexp_k logits - (a[b,j] + t[b,j+2]) )

Since a[b,j] is constant over k, logsumexp_k(a+t) = a + logsumexp_k(t), so the
a term (the whole LSTM + fc_b path) cancels exactly:
    loss = mean_{b,j}[ log(sum_{k>=j+2} exp(t[b,k])) - t[b,j+2] ]

exp() is safe without max-subtraction here: |t| <= ~6 for any plausible input
scale (t is a 1024-dim dot with weights ~U(+-1/sqrt(1536))).

Per-core program (data-parallel over batch, 2 batches/core on 8 cores).
Every DMA costs ~2.2-3.3us END TO END in fixed overhead (descriptor
generation + DGE start delay + completion-semaphore propagation); transfer
time is secondary at these sizes. Structure chosen from HW measurements:

  1. "mini" DMA (critical path, SP queue): [64 x 63] tensor with the gather
     offsets (int32 bit-cast into f32 storage), the 0/1 suffix masks and
     zeros used as activation biases. 252B rows -> fast transfer.
  2. "wtst" DMA (ACT queue, fully hidden behind 1): w_t replicated to the
     64 gather partitions.
  3. Indirect gather of the 64 turn-end rows -> st[64,1024]. 64 x 4KB
     descriptors measured FASTER than 128 x 2KB (HBM random-access latency
     per descriptor), so the full-row layout wins despite idle DVE lanes.
  4. DVE mul + reduce: t[64,1] = sum_E(st * w_t).
  5. PE: C = t . mskC (sum of "correct" logits) concurrent with ACT exp(t).
  6. ACT exp(t) -> e[64,1]; PE: S[1,60] = e^T @ suffix-mask;
     ACT ln(S) with fused row-sum -> rs = sum_j log S_j;
     ACT relu(rs - C) -> per-core partial (each term >= 0 by construction);
     out-DMA from the SP queue.
Host sums the 8 per-core partials and divides by B*(T-2).

Raw bass with explicit semaphores; every instruction carries at most ONE
sync-wait (walrus rejects multi-wait instruction structs); extra deps are
covered transitively through the wait chain. Same-engine back-to-back
write->read pairs (mul->reduce, ln->relu) need their own semaphore hop:
engines pipeline the next instruction's reads into the previous one's
writeback (race detector confirms).
"""

import numpy as np

B, S, E, T, H = 16, 4096, 1024, 32, 512
NCORES = 8
B_LOC = B // NCORES          # batches per core
R = B_LOC * T                # gathered rows per core (64)
NJ = B_LOC * (T - 2)         # loss terms per core (60)
# mini tensor columns: offsets | suffix mask | correct mask | zeros
MCOLS = 1 + NJ + 1 + 1       # 63

_CACHE = {}


def _build_nc():
    from contextlib import ExitStack

    import concourse.bass as bass
    import concourse.mybir as mybir

    f32 = mybir.dt.float32
    i32 = mybir.dt.int32
    AF = mybir.ActivationFunctionType

    nc = bass.Bass("TRN2", target_bir_lowering=False, debug=False)
    # trim the init preamble: the 4 const-tile memsets and the all-engine
    # barrier are dead weight here (biases come from mini_sb, and our own
    # semaphores express every cross-engine dependency)
    _root = nc.m.functions[0].blocks[0]
    _keep = [
        i
        for i in _root.instructions
        if not (
            type(i).__name__ in ("InstMemset", "InstDrain")
            or i.name.startswith("barrier_")
        )
    ]
    del _root.instructions[:]
    _root.instructions.extend(_keep)

    enc = nc.dram_tensor("enc", [B_LOC * S, E], f32, kind="ExternalInput").ap()
    mini = nc.dram_tensor("mini", [R, MCOLS], f32, kind="ExternalInput").ap()
    wtst = nc.dram_tensor("wtst", [R, E], f32, kind="ExternalInput").ap()
    out = nc.dram_tensor("out", [1, 1], f32, kind="ExternalOutput").ap()

    with ExitStack() as ctx:
        blk = ctx.enter_context(nc.Block())
        s_mini = ctx.enter_context(nc.semaphore("s_mini"))
        s_wt = ctx.enter_context(nc.semaphore("s_wt"))
        s_g = ctx.enter_context(nc.semaphore("s_g"))
        s_w = ctx.enter_context(nc.semaphore("s_w"))
        s_dve = ctx.enter_context(nc.semaphore("s_dve"))
        s_act = ctx.enter_context(nc.semaphore("s_act"))
        s_pe = ctx.enter_context(nc.semaphore("s_pe"))
        s_out = ctx.enter_context(nc.semaphore("s_out"))

        mini_sb = ctx.enter_context(nc.sbuf_tensor("mini_sb", [R, MCOLS], f32))
        st_sb = ctx.enter_context(nc.sbuf_tensor("st_sb", [R, E], f32))
        wt_sb = ctx.enter_context(nc.sbuf_tensor("wt_sb", [R, E], f32))
        prod = ctx.enter_context(nc.sbuf_tensor("prod", [R, E], f32))
        red_sb = ctx.enter_context(nc.sbuf_tensor("red_sb", [R, 1], f32))
        e_sb = ctx.enter_context(nc.sbuf_tensor("e_sb", [R, 1], f32))
        warm_sb = ctx.enter_context(nc.sbuf_tensor("warm_sb", [1, 5], f32))
        lse_sb = ctx.enter_context(nc.sbuf_tensor("lse_sb", [1, NJ], f32))
        rs_sb = ctx.enter_context(nc.sbuf_tensor("rs_sb", [1, 1], f32))
        res_sb = ctx.enter_context(nc.sbuf_tensor("res_sb", [1, 1], f32))
        c_ps = ctx.enter_context(nc.psum_tensor("c_ps", [1, 1], f32))
        s_ps = ctx.enter_context(nc.psum_tensor("s_ps", [1, NJ], f32))

        offs_v = mini_sb[:, :1].bitcast(i32)         # [64,1] gather offsets
        mskS_v = mini_sb[:, 1 : 1 + NJ]              # [64,60] suffix mask
        mskC_v = mini_sb[:, 1 + NJ : 2 + NJ]         # [64,1] correct mask
        zb64_v = mini_sb[:, 2 + NJ : 3 + NJ]         # [64,1] zeros (act bias)
        zb1_v = mini_sb[:1, 2 + NJ : 3 + NJ]         # [1,1] zeros (act bias)

        @blk.sync
        def _(sync):
            sync.dma_start(mini_sb[:], mini).then_inc(s_mini, 16)
            sync.wait_ge(s_act, 3)
            sync.dma_start(out, res_sb[:]).then_inc(s_out, 16)

        @blk.scalar
        def _(scalar):
            scalar.dma_start(wt_sb[:], wtst).then_inc(s_wt, 16)
            # warm the activation tables while the DMAs are in flight
            scalar.wait_ge(s_w, 1)
            scalar.activation(
                out=warm_sb[:, 1:2], in_=warm_sb[:, :1], func=AF.Exp,
                bias=warm_sb[:1, :1],
            )
            scalar.activation(
                out=warm_sb[:, 2:3], in_=warm_sb[:, :1], func=AF.Ln,
                bias=warm_sb[:1, :1],
            )
            scalar.activation(
                out=warm_sb[:, 3:4], in_=warm_sb[:, :1], func=AF.Relu,
                bias=warm_sb[:1, :1],
            )
            scalar.wait_ge(s_dve, 2)
            scalar.activation(
                out=e_sb[:], in_=red_sb[:], func=AF.Exp, bias=zb64_v
            ).then_inc(s_act, 1)
            scalar.wait_ge(s_pe, 2)
            scalar.activation(
                out=lse_sb[:],
                in_=s_ps[:],
                func=AF.Ln,
                bias=zb1_v,
                accum_out=rs_sb[:],
            ).then_inc(s_act, 1)
            scalar.wait_ge(s_act, 2)
            # res = relu(rs - C); every loss term is >= 0 so relu is exact
            scalar.activation(
                out=res_sb[:],
                in_=c_ps[:],
                func=AF.Relu,
                bias=rs_sb[:1, :1],
                scale=-1.0,
            ).then_inc(s_act, 1)

        @blk.gpsimd
        def _(gpsimd):
            gpsimd.wait_ge(s_mini, 16)
            gpsimd.indirect_dma_start(
                out=st_sb[:],
                out_offset=None,
                in_=enc,
                in_offset=bass.IndirectOffsetOnAxis(ap=offs_v[:], axis=0),
            ).then_inc(s_g, 16)

        @blk.vector
        def _(vector):
            vector.memset(warm_sb[:], 1.0).then_inc(s_w, 1)
            vector.wait_ge(s_wt, 16)
            vector.wait_ge(s_g, 16)
            vector.tensor_mul(
                out=prod[:], in0=st_sb[:], in1=wt_sb[:]
            ).then_inc(s_dve, 1)
            vector.wait_ge(s_dve, 1)
            vector.tensor_reduce(
                out=red_sb[:],
                in_=prod[:],
                axis=mybir.AxisListType.X,
                op=mybir.AluOpType.add,
            ).then_inc(s_dve, 1)

        @blk.tensor
        def _(tensor):
            tensor.wait_ge(s_dve, 2)
            # C = t . mskC (correct-logit sum), concurrent with ACT's exp
            tensor.matmul(
                out=c_ps[:], lhsT=red_sb[:], rhs=mskC_v, start=True, stop=True
            ).then_inc(s_pe, 1)
            tensor.wait_ge(s_act, 1)
            tensor.matmul(
                out=s_ps[:], lhsT=e_sb[:, :1], rhs=mskS_v, start=True, stop=True
            ).then_inc(s_pe, 1)

    # trim the end-of-program all-engine barrier (drain + EVSEM butterfly):
    # engines can halt independently; the final drain flushes the out-DMA
    for _b in nc.m.functions[0].blocks:
        if _b.name.endswith("_end"):
            _tail_keep = [
                i
                for i in _b.instructions
                if not (
                    type(i).__name__ == "InstDrain" or i.name.startswith("barrier_")
                )
            ]
            del _b.instructions[:]
            _b.instructions.extend(_tail_keep)
    return nc


def _get_nc():
    if "nc" not in _CACHE:
        _CACHE["nc"] = _build_nc()
    return _CACHE["nc"]


def _build_mini_consts():
    """Constant part of the mini tensor (masks + zeros); col 0 (offsets)
    filled per core."""
    m = np.zeros((R, MCOLS), dtype=np.float32)
    for b in range(B_LOC):
        for k in range(T):
            # suffix mask: msk[b*T+k, b2*(T-2)+j] = (b==b2 and k>=j+2)
            for j in range(T - 2):
                if k >= j + 2:
                    m[b * T + k, 1 + b * (T - 2) + j] = 1.0
            # correct mask: k >= 2
            if k >= 2:
                m[b * T + k, 1 + NJ] = 1.0
    return m


def kernel(encoder_output, his_turn_end_ids, w_ih, w_hh, b_ih, b_hh, fc_w, fc_b):
    from concourse import bass_utils

    nc = _get_nc()
    enc = np.ascontiguousarray(np.asarray(encoder_output, dtype=np.float32))
    ids = np.asarray(his_turn_end_ids)
    w_t = np.asarray(fc_w, dtype=np.float32)[0, H:]  # [E]

    mini = _build_mini_consts()
    wtst = np.ascontiguousarray(
        np.broadcast_to(w_t[None, :], (R, E)).astype(np.float32)
    )

    in_maps = []
    for c in range(NCORES):
        b0 = c * B_LOC
        enc_l = enc[b0 : b0 + B_LOC].reshape(B_LOC * S, E)
        gidx = (
            ids[b0 : b0 + B_LOC].astype(np.int64)
            + (np.arange(B_LOC, dtype=np.int64) * S)[:, None]
        ).reshape(R).astype(np.int32)
        m = mini.copy()
        m[:, 0] = gidx.view(np.float32)
        in_maps.append(
            {"enc": enc_l, "mini": np.ascontiguousarray(m), "wtst": wtst}
        )

    try:
        res = bass_utils.run_bass_kernel_spmd(
            nc, in_maps, core_ids=list(range(NCORES))
        )
    except ModuleNotFoundError:
        # ambient BASS_TRACE with no NTFF hook module on this image --
        # rerun with tracing hard-disabled
        import os

        os.environ["BASS_NEVER_TRACE"] = "1"
        res = bass_utils.run_bass_kernel_spmd(
            nc, in_maps, core_ids=list(range(NCORES))
        )
    _CACHE["last_results"] = res
    total = sum(float(r["out"][0, 0]) for r in res.results)
    return np.float32(total / (B * (T - 2)))


# revision 16
# speedup vs baseline: 1.2599x; 1.2599x over previous
"""Trainium2 Bass kernel for nn_DLI_loss_2 (ragged_sequence).

Reference computation (B=16, S=4096, E=1024, T=32, H=512):
    states[b,k,:] = encoder_output[b, ids[b,k], :]          (ragged gather)
    ... 2-step LSTM -> h2 -> a = h2 @ w_h + fc_b            (per (b,j) scalar)
    t = states @ w_t                                        (w_t = fc_w[0, H:])
    logits[b,j,k] = a[b,j] + t[b,k]  masked to k >= j+2
    loss = mean_j( logsumexp_k logits - (a[b,j] + t[b,j+2]) )

Since a[b,j] is constant over k, logsumexp_k(a+t) = a + logsumexp_k(t), so the
a term (the whole LSTM + fc_b path) cancels exactly:
    loss = mean_{b,j}[ log(sum_{k>=j+2} exp(t[b,k])) - t[b,j+2] ]

exp() is safe without max-subtraction here: |t| <= ~6 for any plausible input
scale (t is a 1024-dim dot with weights ~U(+-1/sqrt(1536))).

Per-core program (data-parallel over batch, 2 batches/core on 8 cores).
Every DMA costs ~2.2-3.3us END TO END in fixed overhead (descriptor
generation + DGE start delay + completion-semaphore propagation); transfer
time is secondary at these sizes. Structure chosen from HW measurements:

  1. "mini" DMA (critical path, SP queue): [64 x 63] tensor with the gather
     offsets (int32 bit-cast into f32 storage), the 0/1 suffix masks and
     zeros used as activation biases. 252B rows -> fast transfer.
  2. "wtd" DMA (ACT queue, fully hidden behind 1): w_t ONCE as [2,512]
     plus two 2x64 selector matrices; the PE broadcasts w_t halves to the
     64 gather partitions in PSUM (wtps0/wtps1 = sel_h^T @ w_t). DMAing
     w_t pre-replicated (64x4KB) measurably delayed the critical-path mini
     DMA: 8 cores x 256KB of redundant input traffic collide in the DMA
     engines/HBM during the input phase.
  3. Indirect gather of the 64 turn-end rows -> st[64,1024]. 64 x 4KB
     descriptors measured FASTER than 128 x 2KB (HBM random-access latency
     per descriptor), so the full-row layout wins despite idle DVE lanes.
  4. DVE fused dot (scalar_tensor_tensor, one pass per w_t half):
     prod = (st * 1.0) * wtps_h, accum_out redA/redB = row sums;
     then t = redA + redB.
  5. PE: C = t . mskC (sum of "correct" logits) concurrent with ACT exp(t).
  6. ACT exp(t) -> e[64,1]; PE: S[1,60] = e^T @ suffix-mask;
     ACT ln(S) with fused row-sum -> rs = sum_j log S_j;
     ACT relu(rs - C) -> per-core partial (each term >= 0 by construction);
     out-DMA from the SP queue.
Host sums the 8 per-core partials and divides by B*(T-2).

Raw bass with explicit semaphores; every instruction carries at most ONE
sync-wait (walrus rejects multi-wait instruction structs); extra deps are
covered transitively through the wait chain. Same-engine back-to-back
write->read pairs (mul->reduce, ln->relu) need their own semaphore hop:
engines pipeline the next instruction's reads into the previous one's
writeback (race detector confirms).
"""

import numpy as np

B, S, E, T, H = 16, 4096, 1024, 32, 512
NCORES = 8
B_LOC = B // NCORES          # batches per core
R = B_LOC * T                # gathered rows per core (64)
NJ = B_LOC * (T - 2)         # loss terms per core (60)
# mini tensor columns: offsets | suffix mask | correct mask | zeros
MCOLS = 1 + NJ + 1 + 1       # 63

_CACHE = {}


def _build_nc():
    from contextlib import ExitStack

    import concourse.bass as bass
    import concourse.mybir as mybir

    f32 = mybir.dt.float32
    i32 = mybir.dt.int32
    AF = mybir.ActivationFunctionType

    nc = bass.Bass("TRN2", target_bir_lowering=False, debug=False)
    # trim the init preamble: the 4 const-tile memsets and the all-engine
    # barrier are dead weight here (biases come from mini_sb, and our own
    # semaphores express every cross-engine dependency)
    _root = nc.m.functions[0].blocks[0]
    _keep = [
        i
        for i in _root.instructions
        if not (
            type(i).__name__ in ("InstMemset", "InstDrain")
            or i.name.startswith("barrier_")
        )
    ]
    del _root.instructions[:]
    _root.instructions.extend(_keep)

    enc = nc.dram_tensor("enc", [B_LOC * S, E], f32, kind="ExternalInput").ap()
    mini = nc.dram_tensor("mini", [R, MCOLS], f32, kind="ExternalInput").ap()
    wtd = nc.dram_tensor("wtd", [2, E // 2 + 2 * R], f32, kind="ExternalInput").ap()
    out = nc.dram_tensor("out", [1, 1], f32, kind="ExternalOutput").ap()

    with ExitStack() as ctx:
        blk = ctx.enter_context(nc.Block())
        s_mini = ctx.enter_context(nc.semaphore("s_mini"))
        s_wtd = ctx.enter_context(nc.semaphore("s_wtd"))
        s_g = ctx.enter_context(nc.semaphore("s_g"))
        s_w = ctx.enter_context(nc.semaphore("s_w"))
        s_dve = ctx.enter_context(nc.semaphore("s_dve"))
        s_act = ctx.enter_context(nc.semaphore("s_act"))
        s_pe = ctx.enter_context(nc.semaphore("s_pe"))
        s_out = ctx.enter_context(nc.semaphore("s_out"))

        mini_sb = ctx.enter_context(nc.sbuf_tensor("mini_sb", [R, MCOLS], f32))
        st_sb = ctx.enter_context(nc.sbuf_tensor("st_sb", [R, E], f32))
        wtd_sb = ctx.enter_context(
            nc.sbuf_tensor("wtd_sb", [2, E // 2 + 2 * R], f32)
        )
        prod = ctx.enter_context(nc.sbuf_tensor("prod", [R, E], f32))
        redA = ctx.enter_context(nc.sbuf_tensor("redA", [R, 1], f32))
        redB = ctx.enter_context(nc.sbuf_tensor("redB", [R, 1], f32))
        t_sb = ctx.enter_context(nc.sbuf_tensor("t_sb", [R, 1], f32))
        e_sb = ctx.enter_context(nc.sbuf_tensor("e_sb", [R, 1], f32))
        warm_sb = ctx.enter_context(nc.sbuf_tensor("warm_sb", [1, 5], f32))
        lse_sb = ctx.enter_context(nc.sbuf_tensor("lse_sb", [1, NJ], f32))
        rs_sb = ctx.enter_context(nc.sbuf_tensor("rs_sb", [1, 1], f32))
        res_sb = ctx.enter_context(nc.sbuf_tensor("res_sb", [1, 1], f32))
        wtps0 = ctx.enter_context(nc.psum_tensor("wtps0", [R, E // 2], f32))
        wtps1 = ctx.enter_context(nc.psum_tensor("wtps1", [R, E // 2], f32))
        c_ps = ctx.enter_context(nc.psum_tensor("c_ps", [1, 1], f32))
        s_ps = ctx.enter_context(nc.psum_tensor("s_ps", [1, NJ], f32))

        offs_v = mini_sb[:, :1].bitcast(i32)         # [64,1] gather offsets
        mskS_v = mini_sb[:, 1 : 1 + NJ]              # [64,60] suffix mask
        mskC_v = mini_sb[:, 1 + NJ : 2 + NJ]         # [64,1] correct mask
        zb64_v = mini_sb[:, 2 + NJ : 3 + NJ]         # [64,1] zeros (act bias)
        zb1_v = mini_sb[:1, 2 + NJ : 3 + NJ]         # [1,1] zeros (act bias)
        EHALF = E // 2
        wtr_v = wtd_sb[:, :EHALF]                    # [2,512] w_t halves
        sel0_v = wtd_sb[:, EHALF : EHALF + R]        # [2,64] selector h=0
        sel1_v = wtd_sb[:, EHALF + R : EHALF + 2 * R]  # [2,64] selector h=1

        @blk.sync
        def _(sync):
            sync.dma_start(mini_sb[:], mini).then_inc(s_mini, 16)
            sync.wait_ge(s_act, 3)
            sync.dma_start(out, res_sb[:]).then_inc(s_out, 16)

        @blk.scalar
        def _(scalar):
            scalar.dma_start(wtd_sb[:], wtd).then_inc(s_wtd, 16)
            # warm the activation tables while the DMAs are in flight
            scalar.wait_ge(s_w, 1)
            scalar.activation(
                out=warm_sb[:, 1:2], in_=warm_sb[:, :1], func=AF.Exp,
                bias=warm_sb[:1, :1],
            )
            scalar.activation(
                out=warm_sb[:, 2:3], in_=warm_sb[:, :1], func=AF.Ln,
                bias=warm_sb[:1, :1],
            )
            scalar.activation(
                out=warm_sb[:, 3:4], in_=warm_sb[:, :1], func=AF.Relu,
                bias=warm_sb[:1, :1],
            )
            scalar.wait_ge(s_dve, 3)
            scalar.activation(
                out=e_sb[:], in_=t_sb[:], func=AF.Exp, bias=zb64_v
            ).then_inc(s_act, 1)
            scalar.wait_ge(s_pe, 4)
            scalar.activation(
                out=lse_sb[:],
                in_=s_ps[:],
                func=AF.Ln,
                bias=zb1_v,
                accum_out=rs_sb[:],
            ).then_inc(s_act, 1)
            scalar.wait_ge(s_act, 2)
            # res = relu(rs - C); every loss term is >= 0 so relu is exact
            scalar.activation(
                out=res_sb[:],
                in_=c_ps[:],
                func=AF.Relu,
                bias=rs_sb[:1, :1],
                scale=-1.0,
            ).then_inc(s_act, 1)

        @blk.gpsimd
        def _(gpsimd):
            gpsimd.wait_ge(s_mini, 16)
            gpsimd.indirect_dma_start(
                out=st_sb[:],
                out_offset=None,
                in_=enc,
                in_offset=bass.IndirectOffsetOnAxis(ap=offs_v[:], axis=0),
            ).then_inc(s_g, 16)

        @blk.vector
        def _(vector):
            vector.memset(warm_sb[:], 1.0).then_inc(s_w, 1)
            vector.wait_ge(s_pe, 2)
            vector.wait_ge(s_g, 16)
            vector.scalar_tensor_tensor(
                out=prod[:, :EHALF],
                in0=st_sb[:, :EHALF],
                scalar=1.0,
                in1=wtps0[:],
                op0=mybir.AluOpType.mult,
                op1=mybir.AluOpType.mult,
                accum_out=redA[:],
            ).then_inc(s_dve, 1)
            vector.scalar_tensor_tensor(
                out=prod[:, EHALF:],
                in0=st_sb[:, EHALF:],
                scalar=1.0,
                in1=wtps1[:],
                op0=mybir.AluOpType.mult,
                op1=mybir.AluOpType.mult,
                accum_out=redB[:],
            ).then_inc(s_dve, 1)
            vector.wait_ge(s_dve, 2)
            vector.tensor_add(
                out=t_sb[:], in0=redA[:], in1=redB[:]
            ).then_inc(s_dve, 1)

        @blk.tensor
        def _(tensor):
            # broadcast w_t halves to 64 partitions: wtps_h = sel_h^T @ wtr
            tensor.wait_ge(s_wtd, 16)
            tensor.matmul(
                out=wtps0[:], lhsT=sel0_v, rhs=wtr_v, start=True, stop=True
            ).then_inc(s_pe, 1)
            tensor.matmul(
                out=wtps1[:], lhsT=sel1_v, rhs=wtr_v, start=True, stop=True
            ).then_inc(s_pe, 1)
            tensor.wait_ge(s_dve, 3)
            # C = t . mskC (correct-logit sum), concurrent with ACT's exp
            tensor.matmul(
                out=c_ps[:], lhsT=t_sb[:], rhs=mskC_v, start=True, stop=True
            ).then_inc(s_pe, 1)
            tensor.wait_ge(s_act, 1)
            tensor.matmul(
                out=s_ps[:], lhsT=e_sb[:, :1], rhs=mskS_v, start=True, stop=True
            ).then_inc(s_pe, 1)

    # trim the end-of-program all-engine barrier (drain + EVSEM butterfly):
    # engines can halt independently; the final drain flushes the out-DMA
    for _b in nc.m.functions[0].blocks:
        if _b.name.endswith("_end"):
            _tail_keep = [
                i
                for i in _b.instructions
                if not (
                    type(i).__name__ == "InstDrain" or i.name.startswith("barrier_")
                )
            ]
            del _b.instructions[:]
            _b.instructions.extend(_tail_keep)
    return nc


def _get_nc():
    if "nc" not in _CACHE:
        _CACHE["nc"] = _build_nc()
    return _CACHE["nc"]


def _build_mini_consts():
    """Constant part of the mini tensor (masks + zeros); col 0 (offsets)
    filled per core."""
    m = np.zeros((R, MCOLS), dtype=np.float32)
    for b in range(B_LOC):
        for k in range(T):
            # suffix mask: msk[b*T+k, b2*(T-2)+j] = (b==b2 and k>=j+2)
            for j in range(T - 2):
                if k >= j + 2:
                    m[b * T + k, 1 + b * (T - 2) + j] = 1.0
            # correct mask: k >= 2
            if k >= 2:
                m[b * T + k, 1 + NJ] = 1.0
    return m


def kernel(encoder_output, his_turn_end_ids, w_ih, w_hh, b_ih, b_hh, fc_w, fc_b):
    from concourse import bass_utils

    nc = _get_nc()
    enc = np.ascontiguousarray(np.asarray(encoder_output, dtype=np.float32))
    ids = np.asarray(his_turn_end_ids)
    w_t = np.asarray(fc_w, dtype=np.float32)[0, H:]  # [E]

    mini = _build_mini_consts()
    EHALF = E // 2
    wtd = np.zeros((2, EHALF + 2 * R), dtype=np.float32)
    wtd[0, :EHALF] = w_t[:EHALF]
    wtd[1, :EHALF] = w_t[EHALF:]
    wtd[0, EHALF : EHALF + R] = 1.0          # sel0: broadcast row 0
    wtd[1, EHALF + R : EHALF + 2 * R] = 1.0  # sel1: broadcast row 1
    wtd = np.ascontiguousarray(wtd)

    in_maps = []
    for c in range(NCORES):
        b0 = c * B_LOC
        enc_l = enc[b0 : b0 + B_LOC].reshape(B_LOC * S, E)
        gidx = (
            ids[b0 : b0 + B_LOC].astype(np.int64)
            + (np.arange(B_LOC, dtype=np.int64) * S)[:, None]
        ).reshape(R).astype(np.int32)
        m = mini.copy()
        m[:, 0] = gidx.view(np.float32)
        in_maps.append(
            {"enc": enc_l, "mini": np.ascontiguousarray(m), "wtd": wtd}
        )

    try:
        res = bass_utils.run_bass_kernel_spmd(
            nc, in_maps, core_ids=list(range(NCORES))
        )
    except ModuleNotFoundError:
        # ambient BASS_TRACE with no NTFF hook module on this image --
        # rerun with tracing hard-disabled
        import os

        os.environ["BASS_NEVER_TRACE"] = "1"
        res = bass_utils.run_bass_kernel_spmd(
            nc, in_maps, core_ids=list(range(NCORES))
        )
    _CACHE["last_results"] = res
    total = sum(float(r["out"][0, 0]) for r in res.results)
    return np.float32(total / (B * (T - 2)))
